# revision 14
# baseline (speedup 1.0000x reference)
"""Chunked causal self-attention with RoPE — Trainium2 Bass/Tile kernel.

Problem: B=4, L=4096, H=16, Dh=Dv=128, chunked (C=1024) causal attention
with rotary embeddings, fp32 inputs/outputs.

Sharding: 8 cores = batch (4) x head-half (2). Each core handles one batch
element and 8 heads: 8 heads x 4 chunks = 32 independent chunk-attention
units of shape (1024, 1024).

Host prep: RoPE is applied on the host (pure elementwise prep, like the
transposes/casts), and Q/K ship as fp8(e4m3) with a small fp16 copy of the
first 128 columns of each chunk (so the few-key rows of q-tile 0 stay
accurate). V ships fp8 (+ ones column for the softmax denominator) plus an
fp16 copy of k-tile 0.

Device per (head, chunk) unit, S^T[k,q] layout (contraction on partitions):
  - S^T tiles via fp8 matmuls; causal masks folded into the PE:
    k-tile0 diagonal gets the exact -500*max(0,k-q) ramp (fp16), other
    diagonals get a binary -160 step (fp8) which drives both exp paths to
    an exact 0 for masked entries.
  - P = exp(alpha*S - 2.5) computed two ways, split across engines for
    balance: ACT exp -> fp8 directly; DVE computes y = a8*S + b8 (fp16)
    and GPSIMD converts int8(max(y,0)) whose bit pattern IS the fp8
    p-value (Schraudolph 2^x in the fp8 exponent/mantissa domain).
  - O[q,dv+1] accumulated with fp8 DoubleRow matmuls (two k-tiles per
    instruction, half PE cost); q-tile 0 uses fp16 P and fp16 V.
  - Raw numerator+denominator evacuate PSUM->SBUF fp16 (split ACT/DVE);
    the host does the final divide in fp32.
"""

import functools
import math
import sys
from concurrent.futures import ThreadPoolExecutor

import numpy as np

if "/opt/trn_rl_repo" not in sys.path:
    sys.path.insert(0, "/opt/trn_rl_repo")

import ml_dtypes

B, L, H, DH, DV = 4, 4096, 16, 128, 128
CHUNK = 1024
NCORES = 8
HPC = H // 2  # heads per core
NPAIR = HPC // 2  # head pairs per core
NCH = L // CHUNK  # chunks
NT = CHUNK // 128  # 128-row k/q tiles per chunk
ROPE_BASE = 10000.0

ALPHA = 1.0 / math.sqrt(DH)
LN2 = math.log(2.0)
EXP_BIAS = -2.5  # p = exp(alpha*s - 2.5); keeps p in fp8 range
# 2^x bit trick: int8(a8*s + b8) bit pattern == fp8e4(exp(alpha*s - 2.5))
A8 = 8.0 * ALPHA / LN2
B8 = 56.0 + 8.0 * EXP_BIAS / LN2 - 8.0 * 0.0430  # Schraudolph centering
MASK_RAMP = -500.0  # k-tile0 diagonal ramp (fp16 path, exact exp->0)
MASK_BIN = -160.0  # other diagonals, binary step (both paths -> exact 0)

# p8 packed layout: [128, 2(member=ki&1), 2560]; pair slot t holds k-tiles
# (2t, 2t+1); block (ki, qi) lives at [ki & 1, M[ki>>1] + 128*qi]. Slot
# origins are packed so member-0 regions tile [128:2560] contiguously.
M_SLOT = [0, 768, 1280, 1536]
P8W = 2560

_PROG_CACHE = {}


def _build_program(cfg=()):
    cfg = dict(cfg)
    from contextlib import ExitStack

    import concourse.bacc as bacc
    import concourse.tile as tile
    from concourse import mybir

    f16 = mybir.dt.float16
    f32 = mybir.dt.float32
    f8 = mybir.dt.float8e4
    i8 = mybir.dt.int8
    MM = mybir.AluOpType
    DR = mybir.MatmulPerfMode.DoubleRow

    nc = bacc.Bacc(None, target_bir_lowering=False)

    # HBM inputs (per core). j = head-in-pair.
    q16_d = nc.dram_tensor("q16", [NPAIR, DH, 2, L], f16, kind="ExternalInput")
    k16_d = nc.dram_tensor("k16", [NPAIR, DH, 2, L], f16, kind="ExternalInput")
    # V fp8 (+ones col, padded to 144): [hp, p, u(all k-tiles), j, 144]
    va8_d = nc.dram_tensor("va8", [NPAIR, 128, NCH * NT, 2, 144], f8, kind="ExternalInput")
    # V fp16 of k-tile0 of each chunk: [hp, p, c, j, 129]
    va16_d = nc.dram_tensor("va16", [NPAIR, 128, NCH, 2, 129], f16, kind="ExternalInput")
    at16_d = nc.dram_tensor("at16", [128, 128], f16, kind="ExternalInput")
    bm16_d = nc.dram_tensor("bm16", [128, 128], f16, kind="ExternalInput")
    id8_d = nc.dram_tensor("id8", [128, 128], f8, kind="ExternalInput")
    bmb8_d = nc.dram_tensor("bmb8", [128, 128], f8, kind="ExternalInput")
    # raw numerator+denominator out: [hp, p, c, qt, j, 129] fp16
    o_d = nc.dram_tensor("o", [NPAIR, 128, NCH, NT, 2, 129], f16, kind="ExternalOutput")

    with tile.TileContext(nc) as tc, ExitStack() as ctx:
        singles = ctx.enter_context(tc.tile_pool(name="singles", bufs=1))
        inp = ctx.enter_context(tc.tile_pool(name="inp", bufs=cfg.get("inp", 3)))
        vp = ctx.enter_context(tc.tile_pool(name="vp", bufs=cfg.get("vp", 3)))
        pp = ctx.enter_context(tc.tile_pool(name="pp", bufs=cfg.get("pp", 2)))
        yp = ctx.enter_context(tc.tile_pool(name="yp", bufs=cfg.get("yp", 3)))
        op = ctx.enter_context(tc.tile_pool(name="op", bufs=cfg.get("op", 2)))
        split = cfg.get("split", 0)
        if split:
            # dedicated ACT-group / DVE-group S pools; smaller O accumulators
            spsA = ctx.enter_context(tc.tile_pool(name="spsA", bufs=2, space="PSUM"))
            spsD = ctx.enter_context(tc.tile_pool(name="spsD", bufs=1, space="PSUM"))
            ops_ = ctx.enter_context(tc.tile_pool(name="ops", bufs=2, space="PSUM"))
        else:
            spsA = spsD = ctx.enter_context(tc.tile_pool(name="sps", bufs=2, space="PSUM"))
            ops_ = ctx.enter_context(tc.tile_pool(name="ops", bufs=2, space="PSUM"))

        at16_t = singles.tile([128, 128], f16)
        nc.sync.dma_start(out=at16_t, in_=at16_d[:, :])
        bm16_t = singles.tile([128, 128], f16)
        nc.sync.dma_start(out=bm16_t, in_=bm16_d[:, :])
        id8_t = singles.tile([128, 128], f8)
        nc.sync.dma_start(out=id8_t, in_=id8_d[:, :])
        bmb8_t = singles.tile([128, 128], f8)
        nc.sync.dma_start(out=bmb8_t, in_=bmb8_d[:, :])
        bias_t = singles.tile([128, 1], f32)
        nc.gpsimd.memset(bias_t[:, :], EXP_BIAS)

        def exp_act(dst_ap, src_ap):
            nc.scalar.activation(
                out=dst_ap, in_=src_ap,
                func=mybir.ActivationFunctionType.Exp,
                scale=ALPHA, bias=bias_t[:, :],
            )

        # Software-pipelined unit schedule: emit S-phase(u) before
        # O-phase(u-1) so PE always has runnable work while the exp/convert
        # chain of the previous unit drains on ACT/DVE/GPSIMD.
        units = []
        for hp in range(NPAIR):
            for c in range(NCH):
                units.append((hp, c))

        pending = []  # emitted S-phase, O-phase not yet emitted

        def s_phase(hp, c, j, tiles):
            (q16_t, k16_t, va8_t, va16_t, out_t) = tiles
            q16j = q16_t[:, j, :]
            k16j = k16_t[:, j, :]
            p8_t = pp.tile([128, 2, P8W], f8, tag="p8")
            p8i = p8_t.bitcast(i8)
            p16_t = pp.tile([128, 128], f16, tag="p16")

            def s_tile(ps, ki, lo, hi, diag16=False):
                """Scores for k-tile ki into psum cols [lo:hi] of ps."""
                if diag16:
                    nc.tensor.matmul(
                        ps[:, lo : lo + 128], lhsT=at16_t, rhs=bm16_t,
                        start=True, stop=False,
                    )
                else:
                    nc.tensor.matmul(
                        ps[:, lo : lo + 128], lhsT=id8_t, rhs=bmb8_t,
                        start=True, stop=False,
                    )
                lhs = k16j[:, ki * 128 : ki * 128 + 128]
                nc.tensor.matmul(
                    ps[:, lo : lo + 128], lhsT=lhs,
                    rhs=q16j[:, ki * 128 : ki * 128 + 128],
                    start=False, stop=True,
                )
                x = lo + 128
                q0 = ki * 128 + 128
                while x < hi:
                    xe = min(hi, (x // 512 + 1) * 512)
                    nc.tensor.matmul(
                        ps[:, x:xe], lhsT=lhs,
                        rhs=q16j[:, q0 + (x - lo - 128) : q0 + (xe - lo - 128)],
                        start=True, stop=True,
                    )
                    x = xe

            def grpA():
                psA = spsA.tile([128, 1024], f32, tag="sA" if split else "s")
                s_tile(psA, 0, 0, 1024, diag16=True)
                exp_act(p16_t[:, :], psA[:, 0:128])
                exp_act(p8_t[:, 0, 128:1024], psA[:, 128:1024])

            def grpB():
                psB = spsA.tile([128, 1024], f32, tag="sA" if split else "s")
                s_tile(psB, 1, 0, 896)
                s_tile(psB, 7, 896, 1024)
                exp_act(p8_t[:, 1, 128:1024], psB[:, 0:896])
                y16B = yp.tile([128, 128], f16, tag="yb")
                nc.vector.tensor_scalar(
                    out=y16B, in0=psB[:, 896:1024], scalar1=A8, scalar2=B8,
                    op0=MM.mult, op1=MM.add,
                )
                nc.gpsimd.tensor_scalar(
                    out=p8i[:, 1, M_SLOT[3] + 896 : M_SLOT[3] + 1024],
                    in0=y16B[:, :],
                    scalar1=0.0, scalar2=None, op0=MM.max,
                )

            def grpC():
                psC = spsA.tile([128, 1024], f32, tag="sA" if split else "s")
                s_tile(psC, 2, 0, 768)
                s_tile(psC, 6, 768, 1024)
                exp_act(p8_t[:, 0, M_SLOT[1] + 256 : M_SLOT[1] + 1024], psC[:, 0:768])
                exp_act(p8_t[:, 0, M_SLOT[3] + 768 : M_SLOT[3] + 1024], psC[:, 768:1024])

            def grpD():
                psD = spsD.tile([128, 1024], f32, tag="sD" if split else "s")
                s_tile(psD, 3, 0, 640)
                s_tile(psD, 5, 640, 1024)
                y16D = yp.tile([128, 1024], f16, tag="y")
                nc.vector.tensor_scalar(
                    out=y16D, in0=psD[:, :], scalar1=A8, scalar2=B8,
                    op0=MM.mult, op1=MM.add,
                )
                nc.gpsimd.tensor_scalar(
                    out=p8i[:, 1, M_SLOT[1] + 384 : M_SLOT[1] + 1024],
                    in0=y16D[:, 0:640],
                    scalar1=0.0, scalar2=None, op0=MM.max,
                )
                nc.gpsimd.tensor_scalar(
                    out=p8i[:, 1, M_SLOT[2] + 640 : M_SLOT[2] + 1024],
                    in0=y16D[:, 640:1024],
                    scalar1=0.0, scalar2=None, op0=MM.max,
                )

            def grpE():
                psE = spsD.tile([128, 1024], f32, tag="sD" if split else "s")
                s_tile(psE, 4, 0, 512)
                y16E = yp.tile([128, 512], f16, tag="ye")
                nc.vector.tensor_scalar(
                    out=y16E, in0=psE[:, 0:512], scalar1=A8, scalar2=B8,
                    op0=MM.mult, op1=MM.add,
                )
                nc.gpsimd.tensor_scalar(
                    out=p8i[:, 0, M_SLOT[2] + 512 : M_SLOT[2] + 1024],
                    in0=y16E[:, :],
                    scalar1=0.0, scalar2=None, op0=MM.max,
                )

            grps = {"A": grpA, "B": grpB, "C": grpC, "D": grpD, "E": grpE}
            for g in cfg.get("order", "ABCDE"):
                grps[g]()
            return (p8_t, p16_t, va8_t, va16_t, out_t, j, hp, c)

        def o_phase(state):
            (p8_t, p16_t, va8_t, va16_t, out_t, j, hp, c) = state
            nacc = 2 if split else 4
            for half in range(8 // nacc):
                o_ps = ops_.tile([128, nacc, 256], f32, tag="ops")
                for qq in range(nacc):
                    qi = half * nacc + qq
                    acc = o_ps[:, qq, 0:129]
                    if qi == 0:
                        nc.tensor.matmul(
                            acc, lhsT=p16_t[:, :], rhs=va16_t[:, j, :],
                            start=True, stop=True,
                        )
                        continue
                    npair = (qi + 1) // 2
                    single = (qi + 1) % 2
                    for t in range(npair):
                        nc.tensor.matmul(
                            acc,
                            lhsT=p8_t[:, :, M_SLOT[t] + 128 * qi : M_SLOT[t] + 128 * qi + 128],
                            rhs=va8_t[:, 2 * t : 2 * t + 2, j, 0:129],
                            start=(t == 0), stop=(single == 0 and t == npair - 1),
                            perf_mode=DR,
                        )
                    if single:
                        nc.tensor.matmul(
                            acc,
                            lhsT=p8_t[:, 0, M_SLOT[qi >> 1] + 128 * qi : M_SLOT[qi >> 1] + 128 * qi + 128],
                            rhs=va8_t[:, qi, j, 0:129],
                            start=False, stop=True,
                        )
                # evacuate raw numerator+denominator (host divides)
                dst = out_t[:, half * nacc : half * nacc + nacc, j, :]
                ev_src = o_ps[:, :, 0:129]
                nc.vector.tensor_copy(dst, ev_src)
            if j == 1:
                nc.sync.dma_start(out=o_d[hp, :, c], in_=out_t)

        for hp, c in units:
            c0 = c * CHUNK
            q16_t = inp.tile([DH, 2, CHUNK], f16, tag="q16")
            nc.sync.dma_start(out=q16_t, in_=q16_d[hp, :, :, c0 : c0 + CHUNK])
            k16_t = inp.tile([DH, 2, CHUNK], f16, tag="k16")
            nc.sync.dma_start(out=k16_t, in_=k16_d[hp, :, :, c0 : c0 + CHUNK])
            va8_t = vp.tile([128, NT, 2, 144], f8, tag="va8")
            nc.sync.dma_start(
                out=va8_t, in_=va8_d[hp, :, c * NT : (c + 1) * NT, :, :]
            )
            va16_t = vp.tile([128, 2, 129], f16, tag="va16")
            nc.sync.dma_start(out=va16_t, in_=va16_d[hp, :, c, :, :])
            out_t = op.tile([128, NT, 2, 129], f16, tag="o")
            tiles = (q16_t, k16_t, va8_t, va16_t, out_t)

            for j in range(2):
                pending.append(s_phase(hp, c, j, tiles))
                if len(pending) > 1:
                    o_phase(pending.pop(0))
        while pending:
            o_phase(pending.pop(0))

    nc.finalize()
    return nc


DEFAULT_CFG = (("split", 1), ("order", "ABDEC"), ("pp", 3))


def _get_program(cfg=None):
    if cfg is None:
        cfg = DEFAULT_CFG
    key = tuple(sorted(dict(cfg).items()))
    if key not in _PROG_CACHE:
        _PROG_CACHE[key] = _build_program(key)
    return _PROG_CACHE[key]


@functools.lru_cache(maxsize=4)
def _rope_tables(start_index):
    half = DH // 2
    freqs = np.exp(np.arange(half, dtype=np.float64) * -(math.log(ROPE_BASE) / half))
    ang = (np.arange(L, dtype=np.float64) + float(start_index))[:, None] * freqs[None, :]
    return np.cos(ang).astype(np.float32), np.sin(ang).astype(np.float32)  # (L, 64)


@functools.lru_cache(maxsize=1)
def _mask_consts():
    j = np.arange(128)
    at16 = (j[:, None] < j[None, :]).astype(np.float16)
    bm16 = np.where(j[:, None] >= j[None, :], np.float16(MASK_RAMP), np.float16(0.0))
    id8 = np.eye(128, dtype=ml_dtypes.float8_e4m3)
    bmb8 = np.where(
        j[:, None] > j[None, :],
        np.float32(MASK_BIN), np.float32(0.0),
    ).astype(ml_dtypes.float8_e4m3)
    return (
        np.ascontiguousarray(at16), np.ascontiguousarray(bm16),
        np.ascontiguousarray(id8), np.ascontiguousarray(bmb8),
    )


def _rope(x, cos, sin):
    # x: (L, nh, 128) fp32 -> rotated, same shape
    x1 = x[..., :64]
    x2 = x[..., 64:]
    c = cos[:, None, :]
    s = sin[:, None, :]
    return np.concatenate([x1 * c - x2 * s, x2 * c + x1 * s], axis=-1)


def _prep_core(q, k, v, start_index, b, hh):
    """Build one core's input map from full fp32 inputs."""
    f8 = ml_dtypes.float8_e4m3
    cos, sin = _rope_tables(start_index)
    qs = q[b, :, hh : hh + HPC, :]  # (L, 8, 128)
    ks = k[b, :, hh : hh + HPC, :]
    qr = _rope(qs, cos, sin)  # (L, 8, 128) fp32
    kr = _rope(ks, cos, sin)

    # q16/k16: [hp, dh, j, L]
    qt = qr.transpose(2, 1, 0)  # (128, 8, L)
    kt = kr.transpose(2, 1, 0)
    q16 = np.ascontiguousarray(
        qt.reshape(DH, NPAIR, 2, L).transpose(1, 0, 2, 3)
    ).astype(np.float16)
    k16 = np.ascontiguousarray(
        kt.reshape(DH, NPAIR, 2, L).transpose(1, 0, 2, 3)
    ).astype(np.float16)

    # va8: [hp, p, u, j, 144] (cols: 128 v, 1 ones, 15 zero-pad)
    vv = v[b, :, hh : hh + HPC, :]  # (L, 8, 128)
    va = np.zeros((NPAIR, 128, NCH * NT, 2, 144), np.float32)
    vr = vv.reshape(NCH * NT, 128, NPAIR, 2, DV)  # (u, p, hp, j, dv)
    va[:, :, :, :, 0:DV] = vr.transpose(2, 1, 0, 3, 4)
    va[:, :, :, :, DV] = 1.0
    va8 = va.astype(f8)

    # va16: [hp, p, c, j, 129] — k-tile0 of each chunk
    va16 = np.ascontiguousarray(
        va[:, :, 0 : NCH * NT : NT, :, 0:129]
    ).astype(np.float16)

    at16, bm16, id8, bmb8 = _mask_consts()
    return {
        "q16": q16, "k16": k16, "va8": va8, "va16": va16,
        "at16": at16, "bm16": bm16, "id8": id8, "bmb8": bmb8,
    }


def _run(in_maps, trace=False):
    from concourse.bass_utils import run_bass_kernel_spmd

    nc = _get_program()
    return run_bass_kernel_spmd(
        nc, in_maps, core_ids=list(range(len(in_maps))), trace=trace
    )


def _finish_core(o, b, hh, out):
    """o: [hp, p, c, qt, j, 129] fp16 raw -> out[b, :, hh:hh+8, :] fp32."""
    of = o.astype(np.float32)
    num = of[..., 0:DV]
    den = of[..., DV : DV + 1]
    r = num / den  # (hp, p, c, qt, j, dv)
    # l = c*1024 + qt*128 + p ; head = hh + 2*hp + j
    r = r.transpose(2, 3, 1, 0, 4, 5)  # (c, qt, p, hp, j, dv)
    out[b, :, hh : hh + HPC, :] = r.reshape(L, HPC, DV)


def kernel(q, k, v, start_index):
    q = np.asarray(q, dtype=np.float32)
    k = np.asarray(k, dtype=np.float32)
    v = np.asarray(v, dtype=np.float32)
    si = float(np.asarray(start_index))

    with ThreadPoolExecutor(max_workers=NCORES) as ex:
        in_maps = list(
            ex.map(
                lambda core: _prep_core(q, k, v, si, core // 2, (core % 2) * HPC),
                range(NCORES),
            )
        )

    res = _run(in_maps)

    out = np.empty((B, L, H, DV), np.float32)
    for core in range(NCORES):
        _finish_core(res.results[core]["o"], core // 2, (core % 2) * HPC, out)
    return out.reshape(B, L, H * DV)


# revision 18
# speedup vs baseline: 1.0657x; 1.0657x over previous
"""Chunked causal self-attention with RoPE — Trainium2 Bass/Tile kernel.

Problem: B=4, L=4096, H=16, Dh=Dv=128, chunked (C=1024) causal attention
with rotary embeddings, fp32 inputs/outputs.

Sharding: 8 cores = batch (4) x head-half (2). Each core handles one batch
element and 8 heads: 8 heads x 4 chunks = 32 independent chunk-attention
units of shape (1024, 1024).

Host prep: RoPE is applied on the host (pure elementwise prep, like the
transposes/casts), and Q/K ship as fp8(e4m3) with a small fp16 copy of the
first 128 columns of each chunk (so the few-key rows of q-tile 0 stay
accurate). V ships fp8 (+ ones column for the softmax denominator) plus an
fp16 copy of k-tile 0.

Device per (head, chunk) unit, S^T[k,q] layout (contraction on partitions):
  - S^T tiles via fp8 matmuls; causal masks folded into the PE:
    k-tile0 diagonal gets the exact -500*max(0,k-q) ramp (fp16), other
    diagonals get a binary -160 step (fp8) which drives both exp paths to
    an exact 0 for masked entries.
  - P = exp(alpha*S - 2.5) computed two ways, split across engines for
    balance: ACT exp -> fp8 directly; DVE computes y = a8*S + b8 (fp16)
    and GPSIMD converts int8(max(y,0)) whose bit pattern IS the fp8
    p-value (Schraudolph 2^x in the fp8 exponent/mantissa domain).
  - O[q,dv+1] accumulated with fp8 DoubleRow matmuls (two k-tiles per
    instruction, half PE cost); q-tile 0 uses fp16 P and fp16 V.
  - Raw numerator+denominator evacuate PSUM->SBUF fp16 (split ACT/DVE);
    the host does the final divide in fp32.
"""

import functools
import math
import sys
from concurrent.futures import ThreadPoolExecutor

import numpy as np

if "/opt/trn_rl_repo" not in sys.path:
    sys.path.insert(0, "/opt/trn_rl_repo")

import ml_dtypes

B, L, H, DH, DV = 4, 4096, 16, 128, 128
CHUNK = 1024
NCORES = 8
HPC = H // 2  # heads per core
NPAIR = HPC // 2  # head pairs per core
NCH = L // CHUNK  # chunks
NT = CHUNK // 128  # 128-row k/q tiles per chunk
ROPE_BASE = 10000.0

ALPHA = 1.0 / math.sqrt(DH)
LN2 = math.log(2.0)
EXP_BIAS = -2.5  # p = exp(alpha*s - 2.5); keeps p in fp8 range
# 2^x bit trick: int8(a8*s + b8) bit pattern == fp8e4(exp(alpha*s - 2.5))
A8 = 8.0 * ALPHA / LN2
B8 = 56.0 + 8.0 * EXP_BIAS / LN2 - 8.0 * 0.0430  # Schraudolph centering
MASK_RAMP = -500.0  # k-tile0 diagonal ramp (fp16 path, exact exp->0)
MASK_BIN = -160.0  # other diagonals, binary step (both paths -> exact 0)

# p8 packed layout: [128, 2(member=ki&1), 2560]; pair slot t holds k-tiles
# (2t, 2t+1); block (ki, qi) lives at [ki & 1, M[ki>>1] + 128*qi]. Slot
# origins are packed so member-0 regions tile [128:2560] contiguously.
M_SLOT = [0, 768, 1536, 1024]
P8W = 2560

_PROG_CACHE = {}


def _build_program(cfg=()):
    cfg = dict(cfg)
    from contextlib import ExitStack

    import concourse.bacc as bacc
    import concourse.tile as tile
    from concourse import mybir

    f16 = mybir.dt.float16
    f32 = mybir.dt.float32
    f8 = mybir.dt.float8e4
    i8 = mybir.dt.int8
    MM = mybir.AluOpType
    DR = mybir.MatmulPerfMode.DoubleRow

    nc = bacc.Bacc(None, target_bir_lowering=False)

    # HBM inputs (per core). j = head-in-pair.
    q16_d = nc.dram_tensor("q16", [NPAIR, DH, 2, L], f16, kind="ExternalInput")
    k16_d = nc.dram_tensor("k16", [NPAIR, DH, 2, L], f16, kind="ExternalInput")
    # V fp8 (+ones col, padded to 144): [hp, p, u(all k-tiles), j, 144]
    va8_d = nc.dram_tensor("va8", [NPAIR, 128, NCH * NT, 2, 144], f8, kind="ExternalInput")
    # V fp16 of k-tile0 of each chunk: [hp, p, c, j, 129]
    va16_d = nc.dram_tensor("va16", [NPAIR, 128, NCH, 2, 129], f16, kind="ExternalInput")
    at16_d = nc.dram_tensor("at16", [128, 128], f16, kind="ExternalInput")
    bm16_d = nc.dram_tensor("bm16", [128, 128], f16, kind="ExternalInput")
    id8_d = nc.dram_tensor("id8", [128, 128], f8, kind="ExternalInput")
    bmb8_d = nc.dram_tensor("bmb8", [128, 128], f8, kind="ExternalInput")
    # raw numerator+denominator out: [hp, p, c, qt, j, 129] fp16
    o_d = nc.dram_tensor("o", [NPAIR, 128, NCH, NT, 2, 129], f16, kind="ExternalOutput")

    with tile.TileContext(nc) as tc, ExitStack() as ctx:
        singles = ctx.enter_context(tc.tile_pool(name="singles", bufs=1))
        inp = ctx.enter_context(tc.tile_pool(name="inp", bufs=cfg.get("inp", 3)))
        vp = ctx.enter_context(tc.tile_pool(name="vp", bufs=cfg.get("vp", 3)))
        pp = ctx.enter_context(tc.tile_pool(name="pp", bufs=cfg.get("pp", 2)))
        yp = ctx.enter_context(tc.tile_pool(name="yp", bufs=cfg.get("yp", 3)))
        op = ctx.enter_context(tc.tile_pool(name="op", bufs=cfg.get("op", 2)))
        split = cfg.get("split", 0)
        if split:
            # dedicated ACT-group / DVE-group S pools; smaller O accumulators
            spsA = ctx.enter_context(tc.tile_pool(name="spsA", bufs=2, space="PSUM"))
            spsD = ctx.enter_context(tc.tile_pool(name="spsD", bufs=1, space="PSUM"))
            ops_ = ctx.enter_context(tc.tile_pool(name="ops", bufs=1 if cfg.get("nacc4") else 2, space="PSUM"))
        else:
            spsA = spsD = ctx.enter_context(tc.tile_pool(name="sps", bufs=2, space="PSUM"))
            ops_ = ctx.enter_context(tc.tile_pool(name="ops", bufs=2, space="PSUM"))

        at16_t = singles.tile([128, 128], f16)
        nc.sync.dma_start(out=at16_t, in_=at16_d[:, :])
        bm16_t = singles.tile([128, 128], f16)
        nc.sync.dma_start(out=bm16_t, in_=bm16_d[:, :])
        id8_t = singles.tile([128, 128], f8)
        nc.sync.dma_start(out=id8_t, in_=id8_d[:, :])
        bmb8_t = singles.tile([128, 128], f8)
        nc.sync.dma_start(out=bmb8_t, in_=bmb8_d[:, :])
        bias_t = singles.tile([128, 1], f32)
        nc.gpsimd.memset(bias_t[:, :], EXP_BIAS)

        def exp_act(dst_ap, src_ap):
            nc.scalar.activation(
                out=dst_ap, in_=src_ap,
                func=mybir.ActivationFunctionType.Exp,
                scale=ALPHA, bias=bias_t[:, :],
            )

        # Software-pipelined unit schedule: emit S-phase(u) before
        # O-phase(u-1) so PE always has runnable work while the exp/convert
        # chain of the previous unit drains on ACT/DVE/GPSIMD.
        units = []
        for hp in range(NPAIR):
            for c in range(NCH):
                units.append((hp, c))

        pending = []  # emitted S-phase, O-phase not yet emitted

        def s_phase(hp, c, j, tiles):
            (q16_t, k16_t, va8_t, va16_t, out_t) = tiles
            q16j = q16_t[:, j, :]
            k16j = k16_t[:, j, :]
            p8_t = pp.tile([128, 2, P8W], f8, tag="p8")
            p8i = p8_t.bitcast(i8)
            p16_t = pp.tile([128, 128], f16, tag="p16")

            def s_tile(ps, ki, lo, hi, diag16=False):
                """Scores for k-tile ki into psum cols [lo:hi] of ps."""
                if diag16:
                    nc.tensor.matmul(
                        ps[:, lo : lo + 128], lhsT=at16_t, rhs=bm16_t,
                        start=True, stop=False,
                    )
                else:
                    nc.tensor.matmul(
                        ps[:, lo : lo + 128], lhsT=id8_t, rhs=bmb8_t,
                        start=True, stop=False,
                    )
                lhs = k16j[:, ki * 128 : ki * 128 + 128]
                nc.tensor.matmul(
                    ps[:, lo : lo + 128], lhsT=lhs,
                    rhs=q16j[:, ki * 128 : ki * 128 + 128],
                    start=False, stop=True,
                )
                x = lo + 128
                q0 = ki * 128 + 128
                while x < hi:
                    xe = min(hi, (x // 512 + 1) * 512)
                    nc.tensor.matmul(
                        ps[:, x:xe], lhsT=lhs,
                        rhs=q16j[:, q0 + (x - lo - 128) : q0 + (xe - lo - 128)],
                        start=True, stop=True,
                    )
                    x = xe

            def grpA():
                psA = spsA.tile([128, 1024], f32, tag="sA" if split else "s")
                s_tile(psA, 0, 0, 1024, diag16=True)
                exp_act(p16_t[:, :], psA[:, 0:128])
                exp_act(p8_t[:, 0, 128:1024], psA[:, 128:1024])

            def grpB():
                psB = spsA.tile([128, 1024], f32, tag="sA" if split else "s")
                s_tile(psB, 1, 0, 896)
                exp_act(p8_t[:, 1, 128:1024], psB[:, 0:896])
                if not cfg.get("e7"):
                    s_tile(psB, 7, 896, 1024)
                    y16B = yp.tile([128, 128], f16, tag="yb")
                    nc.vector.tensor_scalar(
                        out=y16B, in0=psB[:, 896:1024], scalar1=A8, scalar2=B8,
                        op0=MM.mult, op1=MM.add,
                    )
                    nc.gpsimd.tensor_scalar(
                        out=p8i[:, 1, M_SLOT[3] + 896 : M_SLOT[3] + 1024],
                        in0=y16B[:, :],
                        scalar1=0.0, scalar2=None, op0=MM.max,
                    )

            def grpC():
                psC = spsA.tile([128, 1024], f32, tag="sA" if split else "s")
                s_tile(psC, 2, 0, 768)
                s_tile(psC, 6, 768, 1024)
                # ki2 then ki6 tile p8 member-0 contiguously: one exp op
                exp_act(p8_t[:, 0, M_SLOT[1] + 256 : M_SLOT[3] + 1024], psC[:, 0:1024])

            def grpD():
                psD = spsD.tile([128, 1024], f32, tag="sD" if split else "s")
                s_tile(psD, 3, 0, 640)
                s_tile(psD, 5, 640, 1024)
                y16D = yp.tile([128, 1024], f16, tag="y")
                nc.vector.tensor_scalar(
                    out=y16D, in0=psD[:, :], scalar1=A8, scalar2=B8,
                    op0=MM.mult, op1=MM.add,
                )
                nc.gpsimd.tensor_scalar(
                    out=p8i[:, 1, M_SLOT[1] + 384 : M_SLOT[1] + 1024],
                    in0=y16D[:, 0:640],
                    scalar1=0.0, scalar2=None, op0=MM.max,
                )
                nc.gpsimd.tensor_scalar(
                    out=p8i[:, 1, M_SLOT[2] + 640 : M_SLOT[2] + 1024],
                    in0=y16D[:, 640:1024],
                    scalar1=0.0, scalar2=None, op0=MM.max,
                )

            def grpE():
                psE = spsD.tile([128, 1024], f32, tag="sD" if split else "s")
                s_tile(psE, 4, 0, 512)
                ew = 512
                if cfg.get("e7"):
                    s_tile(psE, 7, 512, 640)
                    ew = 640
                y16E = yp.tile([128, 640], f16, tag="ye")
                nc.vector.tensor_scalar(
                    out=y16E[:, 0:ew], in0=psE[:, 0:ew], scalar1=A8, scalar2=B8,
                    op0=MM.mult, op1=MM.add,
                )
                nc.gpsimd.tensor_scalar(
                    out=p8i[:, 0, M_SLOT[2] + 512 : M_SLOT[2] + 1024],
                    in0=y16E[:, 0:512],
                    scalar1=0.0, scalar2=None, op0=MM.max,
                )
                if cfg.get("e7"):
                    nc.gpsimd.tensor_scalar(
                        out=p8i[:, 1, M_SLOT[3] + 896 : M_SLOT[3] + 1024],
                        in0=y16E[:, 512:640],
                        scalar1=0.0, scalar2=None, op0=MM.max,
                    )

            grps = {"A": grpA, "B": grpB, "C": grpC, "D": grpD, "E": grpE}
            for g in cfg.get("order", "ABCDE"):
                grps[g]()
            return (p8_t, p16_t, va8_t, va16_t, out_t, j, hp, c)

        def o_phase(state):
            (p8_t, p16_t, va8_t, va16_t, out_t, j, hp, c) = state
            nacc = (4 if cfg.get("nacc4") else 2) if split else 4
            for half in range(8 // nacc):
                o_ps = ops_.tile([128, nacc, 256], f32, tag="ops")
                for qq in range(nacc):
                    qi = half * nacc + qq
                    acc = o_ps[:, qq, 0:129]
                    if qi == 0:
                        nc.tensor.matmul(
                            acc, lhsT=p16_t[:, :], rhs=va16_t[:, j, :],
                            start=True, stop=True,
                        )
                        continue
                    npair = (qi + 1) // 2
                    single = (qi + 1) % 2
                    for t in range(npair):
                        nc.tensor.matmul(
                            acc,
                            lhsT=p8_t[:, :, M_SLOT[t] + 128 * qi : M_SLOT[t] + 128 * qi + 128],
                            rhs=va8_t[:, 2 * t : 2 * t + 2, j, 0:129],
                            start=(t == 0), stop=(single == 0 and t == npair - 1),
                            perf_mode=DR,
                        )
                    if single:
                        nc.tensor.matmul(
                            acc,
                            lhsT=p8_t[:, 0, M_SLOT[qi >> 1] + 128 * qi : M_SLOT[qi >> 1] + 128 * qi + 128],
                            rhs=va8_t[:, qi, j, 0:129],
                            start=False, stop=True,
                        )
                # evacuate raw numerator+denominator (host divides)
                dst = out_t[:, half * nacc : half * nacc + nacc, j, :]
                ev_src = o_ps[:, :, 0:129]
                evprio = cfg.get("evprio", 0)
                if evprio:
                    with tc.high_priority(offset=evprio):
                        nc.vector.tensor_copy(dst, ev_src)
                else:
                    nc.vector.tensor_copy(dst, ev_src)
            if j == 1:
                nc.sync.dma_start(out=o_d[hp, :, c], in_=out_t)

        for hp, c in units:
            c0 = c * CHUNK
            q16_t = inp.tile([DH, 2, CHUNK], f16, tag="q16")
            nc.sync.dma_start(out=q16_t, in_=q16_d[hp, :, :, c0 : c0 + CHUNK])
            k16_t = inp.tile([DH, 2, CHUNK], f16, tag="k16")
            nc.sync.dma_start(out=k16_t, in_=k16_d[hp, :, :, c0 : c0 + CHUNK])
            va8_t = vp.tile([128, NT, 2, 144], f8, tag="va8")
            nc.sync.dma_start(
                out=va8_t, in_=va8_d[hp, :, c * NT : (c + 1) * NT, :, :]
            )
            va16_t = vp.tile([128, 2, 129], f16, tag="va16")
            nc.sync.dma_start(out=va16_t, in_=va16_d[hp, :, c, :, :])
            out_t = op.tile([128, NT, 2, 129], f16, tag="o")
            tiles = (q16_t, k16_t, va8_t, va16_t, out_t)

            for j in range(2):
                pending.append(s_phase(hp, c, j, tiles))
                if len(pending) > cfg.get("lag", 1):
                    o_phase(pending.pop(0))
        while pending:
            o_phase(pending.pop(0))

    nc.finalize()
    return nc


DEFAULT_CFG = (("split", 1), ("order", "AEDBC"), ("pp", 3), ("e7", 1))


def _get_program(cfg=None):
    if cfg is None:
        cfg = DEFAULT_CFG
    key = tuple(sorted(dict(cfg).items()))
    if key not in _PROG_CACHE:
        _PROG_CACHE[key] = _build_program(key)
    return _PROG_CACHE[key]


@functools.lru_cache(maxsize=4)
def _rope_tables(start_index):
    half = DH // 2
    freqs = np.exp(np.arange(half, dtype=np.float64) * -(math.log(ROPE_BASE) / half))
    ang = (np.arange(L, dtype=np.float64) + float(start_index))[:, None] * freqs[None, :]
    return np.cos(ang).astype(np.float32), np.sin(ang).astype(np.float32)  # (L, 64)


@functools.lru_cache(maxsize=1)
def _mask_consts():
    j = np.arange(128)
    at16 = (j[:, None] < j[None, :]).astype(np.float16)
    bm16 = np.where(j[:, None] >= j[None, :], np.float16(MASK_RAMP), np.float16(0.0))
    id8 = np.eye(128, dtype=ml_dtypes.float8_e4m3)
    bmb8 = np.where(
        j[:, None] > j[None, :],
        np.float32(MASK_BIN), np.float32(0.0),
    ).astype(ml_dtypes.float8_e4m3)
    return (
        np.ascontiguousarray(at16), np.ascontiguousarray(bm16),
        np.ascontiguousarray(id8), np.ascontiguousarray(bmb8),
    )


def _rope(x, cos, sin):
    # x: (L, nh, 128) fp32 -> rotated, same shape
    x1 = x[..., :64]
    x2 = x[..., 64:]
    c = cos[:, None, :]
    s = sin[:, None, :]
    return np.concatenate([x1 * c - x2 * s, x2 * c + x1 * s], axis=-1)


def _prep_core(q, k, v, start_index, b, hh):
    """Build one core's input map from full fp32 inputs."""
    f8 = ml_dtypes.float8_e4m3
    cos, sin = _rope_tables(start_index)
    qs = q[b, :, hh : hh + HPC, :]  # (L, 8, 128)
    ks = k[b, :, hh : hh + HPC, :]
    qr = _rope(qs, cos, sin)  # (L, 8, 128) fp32
    kr = _rope(ks, cos, sin)

    # q16/k16: [hp, dh, j, L]
    qt = qr.transpose(2, 1, 0)  # (128, 8, L)
    kt = kr.transpose(2, 1, 0)
    q16 = np.ascontiguousarray(
        qt.reshape(DH, NPAIR, 2, L).transpose(1, 0, 2, 3)
    ).astype(np.float16)
    k16 = np.ascontiguousarray(
        kt.reshape(DH, NPAIR, 2, L).transpose(1, 0, 2, 3)
    ).astype(np.float16)

    # va8: [hp, p, u, j, 144] (cols: 128 v, 1 ones, 15 zero-pad)
    vv = v[b, :, hh : hh + HPC, :]  # (L, 8, 128)
    va = np.zeros((NPAIR, 128, NCH * NT, 2, 144), np.float32)
    vr = vv.reshape(NCH * NT, 128, NPAIR, 2, DV)  # (u, p, hp, j, dv)
    va[:, :, :, :, 0:DV] = vr.transpose(2, 1, 0, 3, 4)
    va[:, :, :, :, DV] = 1.0
    va8 = va.astype(f8)

    # va16: [hp, p, c, j, 129] — k-tile0 of each chunk
    va16 = np.ascontiguousarray(
        va[:, :, 0 : NCH * NT : NT, :, 0:129]
    ).astype(np.float16)

    at16, bm16, id8, bmb8 = _mask_consts()
    return {
        "q16": q16, "k16": k16, "va8": va8, "va16": va16,
        "at16": at16, "bm16": bm16, "id8": id8, "bmb8": bmb8,
    }


def _run(in_maps, trace=False):
    from concourse.bass_utils import run_bass_kernel_spmd

    nc = _get_program()
    return run_bass_kernel_spmd(
        nc, in_maps, core_ids=list(range(len(in_maps))), trace=trace
    )


def _finish_core(o, b, hh, out):
    """o: [hp, p, c, qt, j, 129] fp16 raw -> out[b, :, hh:hh+8, :] fp32."""
    of = o.astype(np.float32)
    num = of[..., 0:DV]
    den = of[..., DV : DV + 1]
    r = num / den  # (hp, p, c, qt, j, dv)
    # l = c*1024 + qt*128 + p ; head = hh + 2*hp + j
    r = r.transpose(2, 3, 1, 0, 4, 5)  # (c, qt, p, hp, j, dv)
    out[b, :, hh : hh + HPC, :] = r.reshape(L, HPC, DV)


def kernel(q, k, v, start_index):
    q = np.asarray(q, dtype=np.float32)
    k = np.asarray(k, dtype=np.float32)
    v = np.asarray(v, dtype=np.float32)
    si = float(np.asarray(start_index))

    with ThreadPoolExecutor(max_workers=NCORES) as ex:
        in_maps = list(
            ex.map(
                lambda core: _prep_core(q, k, v, si, core // 2, (core % 2) * HPC),
                range(NCORES),
            )
        )

    res = _run(in_maps)

    out = np.empty((B, L, H, DV), np.float32)
    for core in range(NCORES):
        _finish_core(res.results[core]["o"], core // 2, (core % 2) * HPC, out)
    return out.reshape(B, L, H * DV)


# revision 19
# speedup vs baseline: 1.0665x; 1.0007x over previous
"""Chunked causal self-attention with RoPE — Trainium2 Bass/Tile kernel.

Problem: B=4, L=4096, H=16, Dh=Dv=128, chunked (C=1024) causal attention
with rotary embeddings, fp32 inputs/outputs.

Sharding: 8 cores = batch (4) x head-half (2). Each core handles one batch
element and 8 heads: 8 heads x 4 chunks = 32 independent chunk-attention
units of shape (1024, 1024).

Host prep: RoPE is applied on the host (pure elementwise prep, like the
transposes/casts), and Q/K ship as fp8(e4m3) with a small fp16 copy of the
first 128 columns of each chunk (so the few-key rows of q-tile 0 stay
accurate). V ships fp8 (+ ones column for the softmax denominator) plus an
fp16 copy of k-tile 0.

Device per (head, chunk) unit, S^T[k,q] layout (contraction on partitions):
  - S^T tiles via fp8 matmuls; causal masks folded into the PE:
    k-tile0 diagonal gets the exact -500*max(0,k-q) ramp (fp16), other
    diagonals get a binary -160 step (fp8) which drives both exp paths to
    an exact 0 for masked entries.
  - P = exp(alpha*S - 2.5) computed two ways, split across engines for
    balance: ACT exp -> fp8 directly; DVE computes y = a8*S + b8 (fp16)
    and GPSIMD converts int8(max(y,0)) whose bit pattern IS the fp8
    p-value (Schraudolph 2^x in the fp8 exponent/mantissa domain).
  - O[q,dv+1] accumulated with fp8 DoubleRow matmuls (two k-tiles per
    instruction, half PE cost); q-tile 0 uses fp16 P and fp16 V.
  - Raw numerator+denominator evacuate PSUM->SBUF fp16 on DVE; the host
    does the final divide in fp32.
"""

import functools
import math
import sys
from concurrent.futures import ThreadPoolExecutor

import numpy as np

if "/opt/trn_rl_repo" not in sys.path:
    sys.path.insert(0, "/opt/trn_rl_repo")

import ml_dtypes

B, L, H, DH, DV = 4, 4096, 16, 128, 128
CHUNK = 1024
NCORES = 8
HPC = H // 2  # heads per core
NPAIR = HPC // 2  # head pairs per core
NCH = L // CHUNK  # chunks
NT = CHUNK // 128  # 128-row k/q tiles per chunk
ROPE_BASE = 10000.0

ALPHA = 1.0 / math.sqrt(DH)
LN2 = math.log(2.0)
EXP_BIAS = -2.5  # p = exp(alpha*s - 2.5); keeps p in fp8 range
# 2^x bit trick: int8(a8*s + b8) bit pattern == fp8e4(exp(alpha*s - 2.5))
A8 = 8.0 * ALPHA / LN2
B8 = 56.0 + 8.0 * EXP_BIAS / LN2 - 8.0 * 0.0430  # Schraudolph centering
MASK_RAMP = -500.0  # k-tile0 diagonal ramp (fp16 path, exact exp->0)
MASK_BIN = -160.0  # other diagonals, binary step (both paths -> exact 0)

# p8 packed layout: [128, 2(member=ki&1), 2560]; pair slot t holds k-tiles
# (2t, 2t+1); block (ki, qi) lives at [ki & 1, M[ki>>1] + 128*qi]. Slot
# origins are packed so member-0 regions tile [128:2560] contiguously.
M_SLOT = [0, 768, 1536, 1024]
P8W = 2560

_PROG_CACHE = {}


def _build_program(cfg=()):
    cfg = dict(cfg)
    from contextlib import ExitStack

    import concourse.bacc as bacc
    import concourse.tile as tile
    from concourse import mybir

    f16 = mybir.dt.float16
    f32 = mybir.dt.float32
    f8 = mybir.dt.float8e4
    i8 = mybir.dt.int8
    MM = mybir.AluOpType
    DR = mybir.MatmulPerfMode.DoubleRow

    nc = bacc.Bacc(None, target_bir_lowering=False)

    # HBM inputs (per core). j = head-in-pair.
    q16_d = nc.dram_tensor("q16", [NPAIR, DH, 2, L], f16, kind="ExternalInput")
    k16_d = nc.dram_tensor("k16", [NPAIR, DH, 2, L], f16, kind="ExternalInput")
    # V fp8 (+ones col, padded to 144): [hp, p, u(all k-tiles), j, 144]
    va8_d = nc.dram_tensor("va8", [NPAIR, 128, NCH * NT, 2, 144], f8, kind="ExternalInput")
    # V fp16 of k-tile0 of each chunk: [hp, p, c, j, 129]
    va16_d = nc.dram_tensor("va16", [NPAIR, 128, NCH, 2, 129], f16, kind="ExternalInput")
    at16_d = nc.dram_tensor("at16", [128, 128], f16, kind="ExternalInput")
    bm16_d = nc.dram_tensor("bm16", [128, 128], f16, kind="ExternalInput")
    id8_d = nc.dram_tensor("id8", [128, 128], f8, kind="ExternalInput")
    bmb8_d = nc.dram_tensor("bmb8", [128, 128], f8, kind="ExternalInput")
    # raw numerator+denominator out: [hp, p, c, qt, j, 129] fp16
    o_d = nc.dram_tensor("o", [NPAIR, 128, NCH, NT, 2, 129], f16, kind="ExternalOutput")

    with tile.TileContext(nc) as tc, ExitStack() as ctx:
        singles = ctx.enter_context(tc.tile_pool(name="singles", bufs=1))
        inp = ctx.enter_context(tc.tile_pool(name="inp", bufs=cfg.get("inp", 3)))
        vp = ctx.enter_context(tc.tile_pool(name="vp", bufs=cfg.get("vp", 3)))
        pp = ctx.enter_context(tc.tile_pool(name="pp", bufs=cfg.get("pp", 2)))
        yp = ctx.enter_context(tc.tile_pool(name="yp", bufs=cfg.get("yp", 3)))
        op = ctx.enter_context(tc.tile_pool(name="op", bufs=cfg.get("op", 2)))
        split = cfg.get("split", 0)
        if split:
            # dedicated ACT-group / DVE-group S pools; smaller O accumulators
            spsA = ctx.enter_context(tc.tile_pool(name="spsA", bufs=2, space="PSUM"))
            spsD = ctx.enter_context(tc.tile_pool(name="spsD", bufs=1, space="PSUM"))
            ops_ = ctx.enter_context(tc.tile_pool(name="ops", bufs=1 if cfg.get("nacc4") else 2, space="PSUM"))
        else:
            spsA = spsD = ctx.enter_context(tc.tile_pool(name="sps", bufs=2, space="PSUM"))
            ops_ = ctx.enter_context(tc.tile_pool(name="ops", bufs=2, space="PSUM"))

        at16_t = singles.tile([128, 128], f16)
        nc.sync.dma_start(out=at16_t, in_=at16_d[:, :])
        bm16_t = singles.tile([128, 128], f16)
        nc.sync.dma_start(out=bm16_t, in_=bm16_d[:, :])
        id8_t = singles.tile([128, 128], f8)
        nc.sync.dma_start(out=id8_t, in_=id8_d[:, :])
        bmb8_t = singles.tile([128, 128], f8)
        nc.sync.dma_start(out=bmb8_t, in_=bmb8_d[:, :])
        bias_t = singles.tile([128, 1], f32)
        nc.gpsimd.memset(bias_t[:, :], EXP_BIAS)

        def exp_act(dst_ap, src_ap):
            nc.scalar.activation(
                out=dst_ap, in_=src_ap,
                func=mybir.ActivationFunctionType.Exp,
                scale=ALPHA, bias=bias_t[:, :],
            )

        # Software-pipelined unit schedule: emit S-phase(u) before
        # O-phase(u-1) so PE always has runnable work while the exp/convert
        # chain of the previous unit drains on ACT/DVE/GPSIMD.
        units = []
        for hp in range(NPAIR):
            for c in range(NCH):
                units.append((hp, c))

        pending = []  # emitted S-phase, O-phase not yet emitted

        def s_phase(hp, c, j, tiles):
            (q16_t, k16_t, va8_t, va16_t, out_t) = tiles
            q16j = q16_t[:, j, :]
            k16j = k16_t[:, j, :]
            p8_t = pp.tile([128, 2, P8W], f8, tag="p8")
            p8i = p8_t.bitcast(i8)
            p16_t = pp.tile([128, 128], f16, tag="p16")

            def s_tile(ps, ki, lo, hi, diag16=False):
                """Scores for k-tile ki into psum cols [lo:hi] of ps."""
                if diag16:
                    nc.tensor.matmul(
                        ps[:, lo : lo + 128], lhsT=at16_t, rhs=bm16_t,
                        start=True, stop=False,
                    )
                else:
                    nc.tensor.matmul(
                        ps[:, lo : lo + 128], lhsT=id8_t, rhs=bmb8_t,
                        start=True, stop=False,
                    )
                lhs = k16j[:, ki * 128 : ki * 128 + 128]
                nc.tensor.matmul(
                    ps[:, lo : lo + 128], lhsT=lhs,
                    rhs=q16j[:, ki * 128 : ki * 128 + 128],
                    start=False, stop=True,
                )
                x = lo + 128
                q0 = ki * 128 + 128
                while x < hi:
                    xe = min(hi, (x // 512 + 1) * 512)
                    nc.tensor.matmul(
                        ps[:, x:xe], lhsT=lhs,
                        rhs=q16j[:, q0 + (x - lo - 128) : q0 + (xe - lo - 128)],
                        start=True, stop=True,
                    )
                    x = xe

            def grpA():
                psA = spsA.tile([128, 1024], f32, tag="sA" if split else "s")
                s_tile(psA, 0, 0, 1024, diag16=True)
                exp_act(p16_t[:, :], psA[:, 0:128])
                exp_act(p8_t[:, 0, 128:1024], psA[:, 128:1024])

            def grpB():
                psB = spsA.tile([128, 1024], f32, tag="sA" if split else "s")
                s_tile(psB, 1, 0, 896)
                exp_act(p8_t[:, 1, 128:1024], psB[:, 0:896])
                if not cfg.get("e7"):
                    s_tile(psB, 7, 896, 1024)
                    y16B = yp.tile([128, 128], f16, tag="yb")
                    nc.vector.tensor_scalar(
                        out=y16B, in0=psB[:, 896:1024], scalar1=A8, scalar2=B8,
                        op0=MM.mult, op1=MM.add,
                    )
                    nc.gpsimd.tensor_scalar(
                        out=p8i[:, 1, M_SLOT[3] + 896 : M_SLOT[3] + 1024],
                        in0=y16B[:, :],
                        scalar1=0.0, scalar2=None, op0=MM.max,
                    )

            def grpC():
                psC = spsA.tile([128, 1024], f32, tag="sA" if split else "s")
                s_tile(psC, 2, 0, 768)
                s_tile(psC, 6, 768, 1024)
                # ki2 then ki6 tile p8 member-0 contiguously: one exp op
                exp_act(p8_t[:, 0, M_SLOT[1] + 256 : M_SLOT[3] + 1024], psC[:, 0:1024])

            def grpD():
                psD = spsD.tile([128, 1024], f32, tag="sD" if split else "s")
                s_tile(psD, 3, 0, 640)
                s_tile(psD, 5, 640, 1024)
                y16D = yp.tile([128, 1024], f16, tag="y")
                nc.vector.tensor_scalar(
                    out=y16D, in0=psD[:, :], scalar1=A8, scalar2=B8,
                    op0=MM.mult, op1=MM.add,
                )
                nc.gpsimd.tensor_scalar(
                    out=p8i[:, 1, M_SLOT[1] + 384 : M_SLOT[1] + 1024],
                    in0=y16D[:, 0:640],
                    scalar1=0.0, scalar2=None, op0=MM.max,
                )
                nc.gpsimd.tensor_scalar(
                    out=p8i[:, 1, M_SLOT[2] + 640 : M_SLOT[2] + 1024],
                    in0=y16D[:, 640:1024],
                    scalar1=0.0, scalar2=None, op0=MM.max,
                )

            def grpE():
                psE = spsD.tile([128, 1024], f32, tag="sD" if split else "s")
                s_tile(psE, 4, 0, 512)
                ew = 512
                if cfg.get("e7"):
                    s_tile(psE, 7, 512, 640)
                    ew = 640
                y16E = yp.tile([128, 640], f16, tag="ye")
                nc.vector.tensor_scalar(
                    out=y16E[:, 0:ew], in0=psE[:, 0:ew], scalar1=A8, scalar2=B8,
                    op0=MM.mult, op1=MM.add,
                )
                nc.gpsimd.tensor_scalar(
                    out=p8i[:, 0, M_SLOT[2] + 512 : M_SLOT[2] + 1024],
                    in0=y16E[:, 0:512],
                    scalar1=0.0, scalar2=None, op0=MM.max,
                )
                if cfg.get("e7"):
                    nc.gpsimd.tensor_scalar(
                        out=p8i[:, 1, M_SLOT[3] + 896 : M_SLOT[3] + 1024],
                        in0=y16E[:, 512:640],
                        scalar1=0.0, scalar2=None, op0=MM.max,
                    )

            grps = {"A": grpA, "B": grpB, "C": grpC, "D": grpD, "E": grpE}
            for g in cfg.get("order", "ABCDE"):
                grps[g]()
            return (p8_t, p16_t, va8_t, va16_t, out_t, j, hp, c)

        def o_phase(state):
            (p8_t, p16_t, va8_t, va16_t, out_t, j, hp, c) = state
            nacc = (4 if cfg.get("nacc4") else 2) if split else 4
            for half in range(8 // nacc):
                o_ps = ops_.tile([128, nacc, 256], f32, tag="ops")
                for qq in range(nacc):
                    qi = half * nacc + qq
                    acc = o_ps[:, qq, 0:129]
                    if qi == 0:
                        nc.tensor.matmul(
                            acc, lhsT=p16_t[:, :], rhs=va16_t[:, j, :],
                            start=True, stop=True,
                        )
                        continue
                    npair = (qi + 1) // 2
                    single = (qi + 1) % 2
                    for t in range(npair):
                        nc.tensor.matmul(
                            acc,
                            lhsT=p8_t[:, :, M_SLOT[t] + 128 * qi : M_SLOT[t] + 128 * qi + 128],
                            rhs=va8_t[:, 2 * t : 2 * t + 2, j, 0:129],
                            start=(t == 0), stop=(single == 0 and t == npair - 1),
                            perf_mode=DR,
                        )
                    if single:
                        nc.tensor.matmul(
                            acc,
                            lhsT=p8_t[:, 0, M_SLOT[qi >> 1] + 128 * qi : M_SLOT[qi >> 1] + 128 * qi + 128],
                            rhs=va8_t[:, qi, j, 0:129],
                            start=False, stop=True,
                        )
                # evacuate raw numerator+denominator (host divides)
                dst = out_t[:, half * nacc : half * nacc + nacc, j, :]
                ev_src = o_ps[:, :, 0:129]
                evprio = cfg.get("evprio", 0)
                if evprio:
                    with tc.high_priority(offset=evprio):
                        nc.vector.tensor_copy(dst, ev_src)
                else:
                    nc.vector.tensor_copy(dst, ev_src)
            if j == 1:
                nc.sync.dma_start(out=o_d[hp, :, c], in_=out_t)

        for hp, c in units:
            c0 = c * CHUNK
            q16_t = inp.tile([DH, 2, CHUNK], f16, tag="q16")
            nc.sync.dma_start(out=q16_t, in_=q16_d[hp, :, :, c0 : c0 + CHUNK])
            k16_t = inp.tile([DH, 2, CHUNK], f16, tag="k16")
            nc.sync.dma_start(out=k16_t, in_=k16_d[hp, :, :, c0 : c0 + CHUNK])
            va8_t = vp.tile([128, NT, 2, 144], f8, tag="va8")
            nc.sync.dma_start(
                out=va8_t, in_=va8_d[hp, :, c * NT : (c + 1) * NT, :, :]
            )
            va16_t = vp.tile([128, 2, 129], f16, tag="va16")
            nc.sync.dma_start(out=va16_t, in_=va16_d[hp, :, c, :, :])
            out_t = op.tile([128, NT, 2, 129], f16, tag="o")
            tiles = (q16_t, k16_t, va8_t, va16_t, out_t)

            for j in range(2):
                pending.append(s_phase(hp, c, j, tiles))
                if len(pending) > cfg.get("lag", 1):
                    o_phase(pending.pop(0))
        while pending:
            o_phase(pending.pop(0))

    nc.finalize()
    return nc


DEFAULT_CFG = (("split", 1), ("order", "AEDBC"), ("pp", 4), ("e7", 1), ("yp", 4), ("op", 3))


def _get_program(cfg=None):
    if cfg is None:
        cfg = DEFAULT_CFG
    key = tuple(sorted(dict(cfg).items()))
    if key not in _PROG_CACHE:
        _PROG_CACHE[key] = _build_program(key)
    return _PROG_CACHE[key]


@functools.lru_cache(maxsize=4)
def _rope_tables(start_index):
    half = DH // 2
    freqs = np.exp(np.arange(half, dtype=np.float64) * -(math.log(ROPE_BASE) / half))
    ang = (np.arange(L, dtype=np.float64) + float(start_index))[:, None] * freqs[None, :]
    return np.cos(ang).astype(np.float32), np.sin(ang).astype(np.float32)  # (L, 64)


@functools.lru_cache(maxsize=1)
def _mask_consts():
    j = np.arange(128)
    at16 = (j[:, None] < j[None, :]).astype(np.float16)
    bm16 = np.where(j[:, None] >= j[None, :], np.float16(MASK_RAMP), np.float16(0.0))
    id8 = np.eye(128, dtype=ml_dtypes.float8_e4m3)
    bmb8 = np.where(
        j[:, None] > j[None, :],
        np.float32(MASK_BIN), np.float32(0.0),
    ).astype(ml_dtypes.float8_e4m3)
    return (
        np.ascontiguousarray(at16), np.ascontiguousarray(bm16),
        np.ascontiguousarray(id8), np.ascontiguousarray(bmb8),
    )


def _rope(x, cos, sin):
    # x: (L, nh, 128) fp32 -> rotated, same shape
    x1 = x[..., :64]
    x2 = x[..., 64:]
    c = cos[:, None, :]
    s = sin[:, None, :]
    return np.concatenate([x1 * c - x2 * s, x2 * c + x1 * s], axis=-1)


def _prep_core(q, k, v, start_index, b, hh):
    """Build one core's input map from full fp32 inputs."""
    f8 = ml_dtypes.float8_e4m3
    cos, sin = _rope_tables(start_index)
    qs = q[b, :, hh : hh + HPC, :]  # (L, 8, 128)
    ks = k[b, :, hh : hh + HPC, :]
    qr = _rope(qs, cos, sin)  # (L, 8, 128) fp32
    kr = _rope(ks, cos, sin)

    # q16/k16: [hp, dh, j, L]
    qt = qr.transpose(2, 1, 0)  # (128, 8, L)
    kt = kr.transpose(2, 1, 0)
    q16 = np.ascontiguousarray(
        qt.reshape(DH, NPAIR, 2, L).transpose(1, 0, 2, 3)
    ).astype(np.float16)
    k16 = np.ascontiguousarray(
        kt.reshape(DH, NPAIR, 2, L).transpose(1, 0, 2, 3)
    ).astype(np.float16)

    # va8: [hp, p, u, j, 144] (cols: 128 v, 1 ones, 15 zero-pad)
    vv = v[b, :, hh : hh + HPC, :]  # (L, 8, 128)
    va = np.zeros((NPAIR, 128, NCH * NT, 2, 144), np.float32)
    vr = vv.reshape(NCH * NT, 128, NPAIR, 2, DV)  # (u, p, hp, j, dv)
    va[:, :, :, :, 0:DV] = vr.transpose(2, 1, 0, 3, 4)
    va[:, :, :, :, DV] = 1.0
    va8 = va.astype(f8)

    # va16: [hp, p, c, j, 129] — k-tile0 of each chunk
    va16 = np.ascontiguousarray(
        va[:, :, 0 : NCH * NT : NT, :, 0:129]
    ).astype(np.float16)

    at16, bm16, id8, bmb8 = _mask_consts()
    return {
        "q16": q16, "k16": k16, "va8": va8, "va16": va16,
        "at16": at16, "bm16": bm16, "id8": id8, "bmb8": bmb8,
    }


def _run(in_maps, trace=False):
    from concourse.bass_utils import run_bass_kernel_spmd

    nc = _get_program()
    return run_bass_kernel_spmd(
        nc, in_maps, core_ids=list(range(len(in_maps))), trace=trace
    )


def _finish_core(o, b, hh, out):
    """o: [hp, p, c, qt, j, 129] fp16 raw -> out[b, :, hh:hh+8, :] fp32."""
    of = o.astype(np.float32)
    num = of[..., 0:DV]
    den = of[..., DV : DV + 1]
    r = num / den  # (hp, p, c, qt, j, dv)
    # l = c*1024 + qt*128 + p ; head = hh + 2*hp + j
    r = r.transpose(2, 3, 1, 0, 4, 5)  # (c, qt, p, hp, j, dv)
    out[b, :, hh : hh + HPC, :] = r.reshape(L, HPC, DV)


def kernel(q, k, v, start_index):
    q = np.asarray(q, dtype=np.float32)
    k = np.asarray(k, dtype=np.float32)
    v = np.asarray(v, dtype=np.float32)
    si = float(np.asarray(start_index))

    with ThreadPoolExecutor(max_workers=NCORES) as ex:
        in_maps = list(
            ex.map(
                lambda core: _prep_core(q, k, v, si, core // 2, (core % 2) * HPC),
                range(NCORES),
            )
        )

    res = _run(in_maps)

    out = np.empty((B, L, H, DV), np.float32)
    for core in range(NCORES):
        _finish_core(res.results[core]["o"], core // 2, (core % 2) * HPC, out)
    return out.reshape(B, L, H * DV)
